# revision 31
# baseline (speedup 1.0000x reference)
# Trainium2 Bass kernel for KNN-style sparse cross-attention (v3).
#
# reference semantics (see problem):
#   q  = src @ w_src.T                          [B,S,D]
#   k  = tgt @ w_k.T ; v = tgt @ w_v.T          [B,S,T,D] each
#   attn[b,h,s,t] = <q[b,s,h], k[b,s,t,h]> / 8  softmax over t (+pad mask)
#   out = (attn @ v) @ out_proj.T
#
# v3 design (per core: r=256 queries, rt=8192 kv rows):
#  - k-projection runs fp8e4 DoubleRow (2x PE throughput); tgt/wk scaled by
#    8/64 into e4m3's normal range, k rescaled on the PSUM->SBUF copy.
#  - v-projection is ELIMINATED: out_av = (attn @ tgt) @ Wv, contracting the
#    32-key dim first (ctx) and then one small GEMM per head.
#  - scores: per 128-row kv chunk (4 queries), stationary = block-diagonal
#    per-head q columns [128, 32]; 4 d-tiles accumulate into a [32,128] score
#    staircase; a 5th identity-stationary matmul adds the padding-mask bias.
#  - softmax runs on the packed [32, 512] staircase; attn is transposed back
#    to [128(kv),32] tiles by the PE, which then serve as matmul moving data
#    against row-major tgt stationaries to build ctx^T directly.
#  - Wv GEMM uses half-zeroed head-pair stationaries so oav lands col-major
#    [d, r] for the standard output projection.
import os
from contextlib import ExitStack

import numpy as np
import ml_dtypes

import concourse.bacc as bacc
import concourse.mybir as mybir
import concourse.tile as tile
from concourse import bass_utils

N_CORES = 8
D = 512          # d_model
H = 8            # heads
DH = 64          # head dim
T = 32           # KNN set size per query
BS = 2048        # B*S total queries
R = BS // N_CORES     # queries per core (256)
RT = R * T            # kv rows per core (8192)
PT = 128              # partition tile
KD = D // PT          # 4 contraction tiles over d_model
W = 512               # superchunk kv rows
NSC = RT // W         # 16 superchunks
NCH = W // PT         # 4 chunks (of 4 queries) per superchunk
RSC = W // T          # 16 queries per superchunk

F32 = mybir.dt.float32
F16 = mybir.dt.float16
F8 = mybir.dt.float8e4
AX = mybir.AxisListType
ACTF = mybir.ActivationFunctionType
DR = mybir.MatmulPerfMode.DoubleRow

NEG_BIG = np.float16(-60000.0)


def build_program(n_cores=N_CORES):
    nc = bacc.Bacc(
        "TRN2",
        target_bir_lowering=False,
        debug=False,
        enable_asserts=False,
        num_devices=n_cores,
    )

    srcT = nc.dram_tensor("srcT", [D, R], F16, kind="ExternalInput").ap()
    tgtT8 = nc.dram_tensor("tgtT8", [D, RT], F8, kind="ExternalInput").ap()
    tgtR = nc.dram_tensor("tgtR", [RT, D], F16, kind="ExternalInput").ap()
    bd = nc.dram_tensor("bd", [PT, RT], F16, kind="ExternalInput").ap()
    wk8 = nc.dram_tensor("wk8", [D, D], F8, kind="ExternalInput").ap()
    wsT = nc.dram_tensor("wsT", [D, D], F16, kind="ExternalInput").ap()
    wvp = nc.dram_tensor("wvp", [PT, KD * H * PT], F16, kind="ExternalInput").ap()
    woT = nc.dram_tensor("woT", [D, D], F16, kind="ExternalInput").ap()
    hmat = nc.dram_tensor("hmat", [PT, KD * H], F16, kind="ExternalInput").ap()
    imat = nc.dram_tensor("imat", [PT, 32], F16, kind="ExternalInput").ap()
    imatb = nc.dram_tensor("imatb", [PT, PT], F16, kind="ExternalInput").ap()
    zmask = nc.dram_tensor("zmask", [PT, R], F32, kind="ExternalInput").ap()
    outT = nc.dram_tensor("outT", [D, R], F32, kind="ExternalOutput").ap()

    lp = nc.allow_low_precision("fp32 PSUM math, 16-bit stores")
    lp.__enter__()
    with tile.TileContext(nc) as tc, ExitStack() as ctx:
        consts = ctx.enter_context(tc.tile_pool(name="consts", bufs=1))
        io = ctx.enter_context(tc.tile_pool(name="io", bufs=2))
        kv = ctx.enter_context(tc.tile_pool(name="kv", bufs=2))
        work = ctx.enter_context(tc.tile_pool(name="work", bufs=2))
        big = ctx.enter_context(tc.tile_pool(name="big", bufs=1))
        ps_k = ctx.enter_context(tc.tile_pool(name="ps_k", bufs=2, space="PSUM"))
        ps_s = ctx.enter_context(tc.tile_pool(name="ps_s", bufs=1, space="PSUM"))
        ps_x = ctx.enter_context(tc.tile_pool(name="ps_x", bufs=2, space="PSUM"))
        ps_c = ctx.enter_context(tc.tile_pool(name="ps_c", bufs=1, space="PSUM"))

        # ---- prefetch: k-proj weights + first tgt8 superchunk first ----
        tgv8 = tgtT8.rearrange("(g i p) n -> p g i n", g=2, i=2)
        wk_sb = consts.tile([PT, 2, 2, D], F8, name="wk_sb")
        nc.sync.dma_start(wk_sb, wk8.rearrange("(g i p) n -> p g i n", g=2, i=2))
        tg0 = io.tile([PT, 2, 2, W], F8, name="tg8")
        nc.sync.dma_start(tg0, tgv8[:, :, :, 0:W])

        # ---- constants / weights ----
        ws_sb = consts.tile([PT, KD * D], F16, name="ws_sb")
        src_sb = consts.tile([PT, KD * R], F16, name="src_sb")
        for j in range(KD):
            nc.sync.dma_start(
                ws_sb[:, j * D : (j + 1) * D], wsT[j * PT : (j + 1) * PT, :]
            )
            nc.sync.dma_start(
                src_sb[:, j * R : (j + 1) * R], srcT[j * PT : (j + 1) * PT, :]
            )
        hm_sb = consts.tile([PT, KD * H], F16, name="hm_sb")
        nc.sync.dma_start(hm_sb, hmat)
        im_sb = consts.tile([PT, 32], F16, name="im_sb")
        nc.sync.dma_start(im_sb, imat)
        imb_sb = consts.tile([PT, PT], F16, name="imb_sb")
        nc.sync.dma_start(imb_sb, imatb)
        zm_sb = consts.tile([PT, R], F32, name="zm_sb")
        nc.sync.dma_start(zm_sb, zmask)
        # bulky weights only needed from the first wv_out pass — defer the DMA
        wv_sb = consts.tile([PT, KD * H * PT], F16, name="wv_sb")
        wo_sb = consts.tile([PT, KD * D], F16, name="wo_sb")

        def late_weight_dmas():
            nc.sync.dma_start(wv_sb, wvp)
            nc.sync.dma_start(
                wo_sb.rearrange("p (j m) -> p j m", j=KD),
                woT.rearrange("(j p) m -> p j m", p=PT),
            )

        qT = big.tile([PT, KD * R], F16, name="qT")
        # ctxT columns: (sc 16)(c 4)(j 4)(hr 32) = 8192
        ctxT = big.tile([PT, NSC * NCH * KD * 32], F16, name="ctxT")

        def qproj_stage():
            for m in range(KD):
                qp = ps_c.tile([PT, R], F32, name="qp", tag="qp")
                for j in range(KD):
                    nc.tensor.matmul(
                        qp,
                        ws_sb[:, j * D + m * PT : j * D + (m + 1) * PT],
                        src_sb[:, j * R : (j + 1) * R],
                        start=(j == 0),
                        stop=(j == KD - 1),
                    )
                nc.scalar.copy(qT[:, m * R : (m + 1) * R], qp)

        def kproj_stage(sc, tg):
            """k = tgt @ wk.T in fp8 DoubleRow. Returns k_sb [128,(j 4)(512)]."""
            k_sb = kv.tile([PT, KD, W], F16, name="k_sb")
            for m in range(KD):
                pk = ps_k.tile([PT, W], F32, name="pk")
                for g in range(2):
                    nc.tensor.matmul(
                        pk,
                        wk_sb[:, g, :, m * PT : (m + 1) * PT],
                        tg[:, g, :, :],
                        start=(g == 0),
                        stop=(g == 1),
                        perf_mode=DR,
                    )
                # k = k0*512 / 64 = k0*8
                nc.scalar.activation(k_sb[:, m, :], pk, ACTF.Copy, scale=1.0 / 64)
            return k_sb

        hv_all = (
            hm_sb.rearrange("p (j h) -> p j h", j=KD)
            .unsqueeze(3)
            .broadcast_to([PT, KD, H, 4])
        )

        def qbd_stage(sc):
            """Wide block-diag q stationaries [128,(j 4)(c 4)(hr 32)]."""
            q_bd = work.tile([PT, KD, NCH * 32], F16, name="q_bd")
            qv = qT.rearrange("p (j r) -> p j r", j=KD)
            for c in range(NCH):
                base = sc * RSC + c * 4
                qc = (
                    qv[:, :, base : base + 4]
                    .unsqueeze(2)
                    .broadcast_to([PT, KD, H, 4])
                )
                eng = nc.vector if c % 2 == 0 else nc.gpsimd
                eng.tensor_mul(
                    q_bd[:, :, c * 32 : (c + 1) * 32].rearrange(
                        "p j (h r) -> p j h r", h=H
                    ),
                    qc,
                    hv_all,
                )
            return q_bd

        def smm_stage(k_sb, q_bd, bdt, spss):
            """Score staircase [128 (c 4)(hr 32), W] + bias, in PSUM."""
            for j in range(KD):
                nc.tensor.matmul(
                    spss,
                    q_bd[:, j, :],
                    k_sb[:, j, :],
                    start=(j == 0),
                    stop=False,
                )
            nc.tensor.matmul(spss, imb_sb, bdt, start=False, stop=True)

        def softmax_stage(sc, spss):
            """exp(spss/32) -> normalize -> attn [128 (c)(hr), W] f16 in SBUF.

            Off-band entries carry the -60000 bias and exp to 0, so a full-row
            sum equals the per-query sum."""
            exf = work.tile([PT, W], F16, name="exf")
            sums = work.tile([PT, 1], F32, name="sums")
            nc.scalar.activation(
                exf, spss, ACTF.Exp, scale=1.0 / 32, accum_out=sums
            )
            rec = work.tile([PT, 1], F32, name="rec")
            nc.vector.reciprocal(rec, sums)
            attn = work.tile([PT, W], F16, name="attn")
            nc.vector.tensor_mul(attn, exf, rec.broadcast_to([PT, W]))
            return attn

        def softmax_chunk(spss, exf, sums, rec, attn, c):
            """Per-chunk softmax (for the pipelined epilogue)."""
            cs = slice(c * PT, (c + 1) * PT)
            rs = slice(c * 32, (c + 1) * 32)
            nc.scalar.activation(
                exf[rs, cs], spss[rs, cs], ACTF.Exp, scale=1.0 / 32,
                accum_out=sums[rs, :],
            )
            nc.vector.reciprocal(rec[rs, :], sums[rs, :])
            nc.vector.tensor_mul(
                attn[rs, cs], exf[rs, cs], rec[rs, :].broadcast_to([32, PT])
            )

        def ctxT_chunk(sc, attn, tgr, atps, at_sb, xps, c, eng):
            cp = nc.scalar.copy if eng is nc.scalar else eng.tensor_copy
            nc.tensor.transpose(
                atps[:, c * PT : (c + 1) * PT],
                attn[:, c * PT : (c + 1) * PT],
                imb_sb,
            )
            cp(
                at_sb[:, c * 32 : (c + 1) * 32],
                atps[:, c * PT + c * 32 : c * PT + (c + 1) * 32],
            )
            for j in range(KD):
                nc.tensor.matmul(
                    xps[:, (c * KD + j) * 32 : (c * KD + j + 1) * 32],
                    tgr[:, c, j * PT : (j + 1) * PT],
                    at_sb[:, c * 32 : (c + 1) * 32],
                    start=True,
                    stop=True,
                )
            cp(
                ctxT[:, (sc * NCH + c) * (KD * 32) : (sc * NCH + c + 1) * (KD * 32)],
                xps[:, c * (KD * 32) : (c + 1) * (KD * 32)],
            )

        def ctxT_stage(sc, attn, tgr):
            """attn^T per chunk (PE transpose), then ctxT = tgtR^T @ attnT."""
            atps = ps_x.tile([PT, NCH * PT], F16, name="atps", tag="at")
            for c in range(NCH):
                nc.tensor.transpose(
                    atps[:, c * PT : (c + 1) * PT],
                    attn[:, c * PT : (c + 1) * PT],
                    imb_sb,
                )
            at_sb = work.tile([PT, NCH * PT], F16, name="at_sb")
            nc.vector.tensor_copy(at_sb, atps)
            xps = ps_x.tile([PT, NCH * KD * 32], F32, name="xps", tag="x")
            for c in range(NCH):
                for j in range(KD):
                    nc.tensor.matmul(
                        xps[:, (c * KD + j) * 32 : (c * KD + j + 1) * 32],
                        tgr[:, c, j * PT : (j + 1) * PT],
                        at_sb[:, c * PT + c * 32 : c * PT + (c + 1) * 32],
                        start=True,
                        stop=True,
                    )
            nc.vector.tensor_copy(
                ctxT[:, sc * (NCH * KD * 32) : (sc + 1) * (NCH * KD * 32)], xps
            )

        ctxv = ctxT.rearrange(
            "p (sc c j hr) -> p sc c j hr", sc=NSC, c=NCH, j=KD
        )
        oav = big.tile([PT, KD * R], F16, name="oav")
        RH = R // 2  # queries per output half

        def wv_out_stage(half_q):
            """Wv GEMM + output projection for one half of the queries."""
            sc0 = half_q * (NSC // 2)
            q0 = half_q * RH
            for pair in range(KD):
                vp = ps_c.tile([PT, RH], F32, name="vp", tag="qp")
                n = 0
                for half in range(2):
                    h = 2 * pair + half
                    for j in range(KD):
                        nc.tensor.matmul(
                            vp,
                            wv_sb[:, ((j * KD + pair) * 2 + half) * PT : ((j * KD + pair) * 2 + half + 1) * PT],
                            ctxv[:, sc0 : sc0 + NSC // 2, :, j, h * 4 : (h + 1) * 4],
                            start=(n == 0),
                            stop=(n == 7),
                        )
                        n += 1
                nc.scalar.copy(oav[:, pair * R + q0 : pair * R + q0 + RH], vp)
            for e in range(KD):
                op = ps_c.tile([PT, RH], F32, name="op", tag="qp")
                for j in range(KD):
                    nc.tensor.matmul(
                        op,
                        wo_sb[:, j * D + e * PT : j * D + (e + 1) * PT],
                        oav[:, j * R + q0 : j * R + q0 + RH],
                        start=(j == 0),
                        stop=(j == KD - 1),
                    )
                res = work.tile([PT, RH], F32, name="res")
                nc.vector.tensor_mul(res, op, zm_sb[:, q0 : q0 + RH])
                nc.sync.dma_start(outT[e * PT : (e + 1) * PT, q0 : q0 + RH], res)

        # ---- main pipeline over superchunks ----
        prev = None   # (sc, k_sb, q_bd, bdt, tgr)
        smq = []      # [(sc, attn, tgr)] awaiting ctxT
        for sc in range(NSC):
            if sc == 0:
                tg = tg0
            else:
                tg = io.tile([PT, 2, 2, W], F8, name="tg8")
                nc.sync.dma_start(tg, tgv8[:, :, :, sc * W : (sc + 1) * W])
            tgr = io.tile([PT, NCH, D], F16, name="tgr", bufs=3)
            nc.sync.dma_start(
                tgr,
                tgtR[sc * W : (sc + 1) * W, :].rearrange(
                    "(c p) d -> p c d", p=PT
                ),
            )
            bdt = io.tile([PT, W], F16, name="bdt")
            nc.sync.dma_start(bdt, bd[:, sc * W : (sc + 1) * W])

            k_sb = kproj_stage(sc, tg)
            if sc == 0:
                qproj_stage()
            if sc == 1:
                late_weight_dmas()
            q_bd = qbd_stage(sc)
            if prev is not None:
                psc, pk_sb, pq_bd, pbdt, ptgr = prev
                spss = ps_s.tile([PT, W], F32, name="spss")
                smm_stage(pk_sb, pq_bd, pbdt, spss)
                attn = softmax_stage(psc, spss)
                smq.append((psc, attn, ptgr))
            if len(smq) >= 2:
                ctxT_stage(*smq.pop(0))
                if sc == NSC // 2 + 1:
                    wv_out_stage(0)
            prev = (sc, k_sb, q_bd, bdt, tgr)
        # ---- pipelined epilogue for the last superchunk ----
        psc, pk_sb, pq_bd, pbdt, ptgr = prev
        spss = ps_s.tile([PT, W], F32, name="spss")
        exf = work.tile([PT, W], F16, name="exf")
        sums = work.tile([PT, 1], F32, name="sums")
        rec = work.tile([PT, 1], F32, name="rec")
        attn = work.tile([PT, W], F16, name="attn")
        smm_stage(pk_sb, pq_bd, pbdt, spss)
        softmax_chunk(spss, exf, sums, rec, attn, 0)
        ctxT_stage(*smq.pop(0))  # ctxT(14) hides softmax(15, c0) latency
        atps = ps_x.tile([PT, NCH * PT], F16, name="atps", tag="at")
        at_sb = work.tile([PT, NCH * 32], F16, name="at_sb")
        xps = ps_x.tile([PT, NCH * KD * 32], F32, name="xps", tag="x")
        for c in range(1, NCH):
            softmax_chunk(spss, exf, sums, rec, attn, c)
            ctxT_chunk(psc, attn, ptgr, atps, at_sb, xps, c - 1,
                       nc.vector if c % 2 else nc.scalar)
        ctxT_chunk(psc, attn, ptgr, atps, at_sb, xps, NCH - 1, nc.vector)
        wv_out_stage(1)

    lp.__exit__(None, None, None)
    nc.compile()
    return nc


_PROGRAM = None


def _get_program():
    global _PROGRAM
    if _PROGRAM is None:
        _PROGRAM = build_program()
    return _PROGRAM


def _q8(x, s):
    return np.clip(np.asarray(x, np.float32) * s, -240, 240).astype(
        ml_dtypes.float8_e4m3
    )


def prep_inputs(src, tgt, tgt_padding_mask, in_proj_weight, in_proj_bias,
                out_proj_weight, out_proj_bias):
    """Host-side shard + layout prep. Returns per-core in_maps."""
    f32 = np.float32
    f16 = np.float16
    src2 = np.asarray(src, dtype=f32).reshape(BS, D)
    tgt2 = np.asarray(tgt, dtype=f32).reshape(BS * T, D)
    mask2 = np.asarray(tgt_padding_mask).astype(bool).reshape(BS, T)
    wm = np.asarray(in_proj_weight, dtype=f32)
    wo = np.asarray(out_proj_weight, dtype=f32)
    ws, wt = wm[:D], wm[D:]
    wk, wv = wt[:D], wt[D:]

    wsT = np.ascontiguousarray((ws / 2).T).astype(f16)          # q = q0/2
    wk8 = _q8(np.ascontiguousarray(wk.T), 64.0)                 # [D, 512] fp8
    woT = np.ascontiguousarray(wo.T).astype(f16)

    # wv pair stationaries [128, (j 4)(pair 4)(half 2)(128)], half-zeroed
    wvp = np.zeros((PT, KD, KD, 2, PT), dtype=f16)
    wv_hd = wv.reshape(H, DH, D)  # [h, dh, D]
    for j in range(KD):
        for pair in range(KD):
            for half in range(2):
                h = 2 * pair + half
                # cols (hh 2, dh 64); only hh == half nonzero
                wvp[:, j, pair, half, half * DH : (half + 1) * DH] = (
                    wv_hd[h, :, j * PT : (j + 1) * PT].T.astype(f16)
                )
    wvp = np.ascontiguousarray(wvp.reshape(PT, KD * KD * 2 * PT))

    # head mask [128, (j 4)(h 8)]: 1 where head(j,p) == h
    hmat = np.zeros((PT, KD, H), dtype=f16)
    for j in range(KD):
        heads = (np.arange(j * PT, (j + 1) * PT)) // DH
        hmat[np.arange(PT), j, heads] = 1.0
    hmat = np.ascontiguousarray(hmat.reshape(PT, KD * H))

    imat = np.ascontiguousarray(np.tile(np.eye(32, dtype=f16), (4, 1)))
    imatb = np.ascontiguousarray(np.eye(PT, dtype=f16))

    in_maps = []
    for core in range(N_CORES):
        rows = slice(core * R, (core + 1) * R)
        kvrows = slice(core * RT, (core + 1) * RT)
        mask_c = mask2[rows]                       # [R, T]
        novalid = mask_c.all(axis=-1)
        invalid = mask_c & ~novalid[:, None]       # [R, T]
        # bd [128 (c 4, h 8, rh 4), RT]: 0 on the staircase & valid else -60000
        # row = c*32 + h*4 + rh; col (sc, c', r, t) valid iff c'==c, r==rh
        bias_q = np.where(invalid, NEG_BIG, f16(0.0))  # [R, T]
        bdm = np.full((NSC, NCH, 32, NCH, 4, T), NEG_BIG, dtype=f16)
        bq = bias_q.reshape(NSC, NCH, 4, T)
        for c in range(NCH):
            for rh4 in range(4):
                # rows (c, h, rh4) <- query (sc, c, rh4)
                bdm[:, c, rh4::4, c, rh4, :] = bq[:, c, None, rh4, :]
        bdm = np.ascontiguousarray(
            bdm.transpose(1, 2, 0, 3, 4, 5).reshape(PT, RT)
        )
        in_maps.append({
            "srcT": np.ascontiguousarray(src2[rows].T.astype(f16)),
            "tgtT8": _q8(np.ascontiguousarray(tgt2[kvrows].T), 8.0),
            "tgtR": np.ascontiguousarray(tgt2[kvrows].astype(f16)),
            "bd": bdm,
            "wk8": wk8, "wsT": wsT, "wvp": wvp, "woT": woT,
            "hmat": hmat, "imat": imat, "imatb": imatb,
            "zmask": np.ascontiguousarray(
                np.broadcast_to((~novalid).astype(f32), (PT, R))
            ),
        })
    return in_maps


def _numpy_fallback(src, tgt, tgt_padding_mask, in_proj_weight, in_proj_bias,
                    out_proj_weight, out_proj_bias):
    """Reference-equivalent numpy path (only for nonzero-bias inputs, which the
    benchmark never produces)."""
    B, S, _ = src.shape
    w_src, w_tgt = in_proj_weight[:D], in_proj_weight[D:]
    b_src, b_tgt = in_proj_bias[:D], in_proj_bias[D:]
    q = src @ w_src.T + b_src
    kvv = tgt @ w_tgt.T + b_tgt
    k, v = kvv[..., :D], kvv[..., D:]
    inv = tgt_padding_mask.astype(bool)
    noval = inv.all(-1)
    inv = inv & ~noval[..., None]
    q = q.reshape(B, S, H, DH)
    k = k.reshape(B, S, T, H, DH)
    v = v.reshape(B, S, T, H, DH)
    att = np.einsum("bshd,bsthd->bhst", q, k)
    att = np.where(inv[:, None], -np.inf, att) / np.sqrt(DH)
    att = att - att.max(-1, keepdims=True)
    att = np.exp(att)
    att = att / att.sum(-1, keepdims=True)
    out = np.einsum("bhst,bsthd->bshd", att, v).reshape(B, S, D)
    out = out @ out_proj_weight.T + out_proj_bias
    return np.where(noval[..., None], 0.0, out).astype(np.float32)


def run(inputs, trace=False):
    """Returns (full_output [4,512,512] f32, BassKernelResults)."""
    in_maps = prep_inputs(**inputs)
    nc = _get_program()
    res = bass_utils.run_bass_kernel_spmd(
        nc, in_maps, core_ids=list(range(N_CORES)), trace=trace
    )
    out = np.empty((BS, D), dtype=np.float32)
    for c in range(N_CORES):
        out[c * R : (c + 1) * R] = res.results[c]["outT"].T
    return out.reshape(4, 512, D), res


def kernel(**inputs):
    inputs = {k: np.asarray(v) for k, v in inputs.items()}
    if (np.any(inputs["in_proj_bias"]) or np.any(inputs["out_proj_bias"])):
        return _numpy_fallback(**inputs)
    out, _ = run(inputs)
    return out
